# revision 59
# baseline (speedup 1.0000x reference)
"""AdaptivePCEN Trainium2 kernel.

Data-parallel over batch: core i computes batches [4i, 4i+4) of the
[32, 128, 4000] input. PPN weights replicated. Per core:
  - PE (bf16): h = relu(W1^T [Xprev; X] + b1), gates = W2^T h + b2,
    laid out so each gate lands as a [F=128, T_chunk] PSUM tile.
  - ACT: sigmoid/exp/ln gate evacuations + PCEN epilogue (no Softplus
    LUT on this toolchain: softplus = ln(1+exp(z))).
  - DVE: tensor_tensor_scan runs the EMA recurrence M_t = (1-s)M +
    s X along the free (time) axis, chained across chunks via the
    per-partition carry.
Matmul accumulation groups must stay inside one 2KB PSUM bank
(bank-crossing output corrupts), so chunks are 1024 cols with
bank-aligned 512-col sub-matmuls (ragged 928 tail).
"""

import numpy as np

B, F, T, H = 32, 128, 4000, 256
N_CORES = 8
BSH = B // N_CORES  # batches per core
CHA = 1024  # phase-A chunk (2 psum banks; subs at 0/512 bank-aligned)
SUBA = 512
PHB = 2  # phase-B pipeline chunks

_COMPILED = {}


def _chunks(t, ch):
    out = []
    t0 = 0
    while t0 < t:
        out.append((t0, min(ch, t - t0)))
        t0 += ch
    return out


def _build(bsh=BSH, t=T, cha=CHA, suba=SUBA, phb=PHB):
    from contextlib import ExitStack

    import concourse.tile as tile
    from concourse import bacc, mybir
    from concourse.tile_rust import add_dep_helper

    f32 = mybir.dt.float32
    bf16 = mybir.dt.bfloat16
    AF = mybir.ActivationFunctionType
    OP = mybir.AluOpType
    EPS = 1e-6

    nc = bacc.Bacc(
        "TRN2", target_bir_lowering=False, debug=False, num_devices=N_CORES
    )

    # X arrives bf16 with the 2-col lead layout already built on the host:
    # col j (j>=2) = X[:, j-2]; col 1 = X[:, 0] (X_prev edge); col 0 pad
    X = nc.dram_tensor("X", [bsh * F, t + 4], bf16, kind="ExternalInput").ap()
    W1 = nc.dram_tensor("W1", [2 * F, H], f32, kind="ExternalInput").ap()
    b1 = nc.dram_tensor("b1", [H, 1], f32, kind="ExternalInput").ap()
    W2 = nc.dram_tensor("W2", [H, 4 * F], f32, kind="ExternalInput").ap()
    b2 = nc.dram_tensor("b2", [4 * F, 1], f32, kind="ExternalInput").ap()
    out = nc.dram_tensor("out", [bsh * F, t], f32, kind="ExternalOutput").ap()

    # phase-B chunk boundaries aligned to phase-A chunk multiples so each
    # gate tile is written by whole phase-A chunks only
    if t > 2 * cha:
        phb_edges = [(0, 2 * cha), (2 * cha, t - 2 * cha)]
    else:
        phb_edges = [(0, t)]
    phb = len(phb_edges)
    tbmax = max(w for _, w in phb_edges)

    with tile.TileContext(nc) as tc, ExitStack() as ctx:
        const = ctx.enter_context(tc.tile_pool(name="const", bufs=1))
        xpool = ctx.enter_context(tc.tile_pool(name="xpool", bufs=2))
        hpsum = ctx.enter_context(tc.tile_pool(name="hpsum", bufs=1, space="PSUM"))
        gpsum = ctx.enter_context(tc.tile_pool(name="gpsum", bufs=2, space="PSUM"))
        hsb = ctx.enter_context(tc.tile_pool(name="hsb", bufs=2))
        gates = ctx.enter_context(tc.tile_pool(name="gates", bufs=2))
        tmp = ctx.enter_context(tc.tile_pool(name="tmp", bufs=1))

        # batch-0 input DMA first (piece 1 covers phase-A chunk 0 + lead)
        xbuf0 = xpool.tile([F, t + 4], bf16, tag="xbuf", name="xbuf_b0")
        nc.sync.dma_start(out=xbuf0[:, 0 : 2 + cha], in_=X[0:F, 0 : 2 + cha])
        nc.sync.dma_start(
            out=xbuf0[:, 2 + cha : t + 4], in_=X[0:F, 2 + cha : t + 4]
        )

        # ---- weights: DMA f32, cast to bf16 ----
        w1f = const.tile([F, 2 * H], f32, tag="w1f")
        nc.sync.dma_start(out=w1f[:, 0:H], in_=W1[0:F, :])
        nc.sync.dma_start(out=w1f[:, H : 2 * H], in_=W1[F : 2 * F, :])
        w1 = const.tile([F, 2 * H], bf16, tag="w1")
        nc.vector.tensor_copy(w1[:], w1f[:])
        w1a = w1[:, 0:H]  # W1 rows 0:F (Xprev part), [K=F, M=H]
        w1b = w1[:, H : 2 * H]  # W1 rows F:2F (X part)

        w2f = const.tile([F, 8 * F], f32, tag="w2f")
        nc.sync.dma_start(out=w2f[:, 0 : 4 * F], in_=W2[0:F, :])
        nc.sync.dma_start(out=w2f[:, 4 * F : 8 * F], in_=W2[F : 2 * F, :])
        w2 = const.tile([F, 8 * F], bf16, tag="w2")
        nc.vector.tensor_copy(w2[:], w2f[:])
        w2a = w2[:, 0 : 4 * F]  # W2 rows 0:H/2 (h1 part), [K, 4F]
        w2b = w2[:, 4 * F : 8 * F]  # W2 rows H/2:H (h2 part)

        bias1 = const.tile([F, 2], f32, tag="bias1")
        nc.sync.dma_start(out=bias1[:, 0:1], in_=b1[0:F, :])
        nc.sync.dma_start(out=bias1[:, 1:2], in_=b1[F : 2 * F, :])
        bias2 = const.tile([F, 4], f32, tag="bias2")
        for g in range(4):
            nc.sync.dma_start(
                out=bias2[:, g : g + 1], in_=b2[g * F : (g + 1) * F, :]
            )
        epsb = const.tile([F, 1], f32, tag="epsb")
        nc.vector.memset(epsb[:], EPS)
        bias2h = const.tile([F, 4], f32, tag="bias2h")
        nc.vector.tensor_scalar(bias2h[:], bias2[:], 0.5, None, OP.mult)

        prev_act = [None]  # last ACT inst of previous batch's chain
        prev_obs = []  # previous batch's ob insts (boundary pull anchors)
        xbuf_tiles = {0: xbuf0}

        def prefetch_x(nb):
            xb = xpool.tile([F, t + 4], bf16, tag="xbuf", name=f"xbuf_{nb}")
            nc.sync.dma_start(
                out=xb[:, 0 : 2 + cha], in_=X[nb * F : (nb + 1) * F, 0 : 2 + cha]
            )
            nc.sync.dma_start(
                out=xb[:, 2 + cha : t + 4],
                in_=X[nb * F : (nb + 1) * F, 2 + cha : t + 4],
            )
            xbuf_tiles[nb] = xb
        NL_SET = 6  # natural_log_exp_and_others in act_info.json

        for b in range(bsh):
            # ---- load X[b], cast to bf16 with 2-col lead layout ----
            # xbuf col j (j>=2) = X[b,:,j-2]; col 1 = X[b,:,0] (X_prev edge)
            # Xcur view = xbuf[:, 2:t+2] (4B aligned), Xprev = xbuf[:, 1:t+1]
            xbuf = xbuf_tiles.pop(b)
            xcur = xbuf[:, 2 : t + 2]

            # gate tiles split per phase-B chunk so chunk 0's scan prep can
            # start while phase A is still filling chunk 1's tiles
            gt = []
            for j, (off, w) in enumerate(phb_edges):
                gt.append({
                    "s": gates.tile([F, w], bf16, tag=f"s{j}", name=f"s_{b}_{j}"),
                    "al": gates.tile([F, w], bf16, tag=f"al{j}", name=f"al_{b}_{j}"),
                    "r": gates.tile([F, w], bf16, tag=f"r{j}", name=f"r_{b}_{j}"),
                })
            E_sb = gates.tile([F, t], bf16, tag="E", name=f"E_{b}")

            sig_insts = []

            for t0, cw in _chunks(t, cha):
                hp1 = hpsum.tile([F, cha], f32, tag="h1")
                hp2 = hpsum.tile([F, cha], f32, tag="h2")
                for s0, sw_ in _chunks(cw, suba):
                    xp = xbuf[:, 1 + t0 + s0 : 1 + t0 + s0 + sw_]
                    xc = xbuf[:, 2 + t0 + s0 : 2 + t0 + s0 + sw_]
                    sl = slice(s0, s0 + sw_)
                    nc.tensor.matmul(hp1[:, sl], w1a[:, 0:F], xp,
                                     start=True, stop=False)
                    nc.tensor.matmul(hp1[:, sl], w1b[:, 0:F], xc,
                                     start=False, stop=True)
                    nc.tensor.matmul(hp2[:, sl], w1a[:, F:H], xp,
                                     start=True, stop=False)
                    nc.tensor.matmul(hp2[:, sl], w1b[:, F:H], xc,
                                     start=False, stop=True)
                h1s = hsb.tile([F, cha], bf16, tag="h1s")
                h2s = hsb.tile([F, cha], bf16, tag="h2s")
                ri1 = nc.vector.tensor_scalar(
                    h1s[:, 0:cw], hp1[:, 0:cw], bias1[:, 0:1], 0.0,
                    OP.add, OP.max,
                )
                ri2 = nc.vector.tensor_scalar(
                    h2s[:, 0:cw], hp2[:, 0:cw], bias1[:, 1:2], 0.0,
                    OP.add, OP.max,
                )
                if t0 == 0 and prev_obs:
                    # pull this batch's first relus ahead of the previous
                    # batch's output-subtract tail in the static DVE order,
                    # so the boundary mm2->tanh chain starts ~5us earlier
                    for oi, ri in zip(prev_obs, (ri1, ri2)):
                        add_dep_helper(oi.ins, ri.ins, sync=True,
                                       reason="boundary relu pull")

                j = next(
                    i for i, (off, w) in enumerate(phb_edges)
                    if off <= t0 < off + w
                )
                joff = t0 - phb_edges[j][0]
                # gates sequentially: u=tanh((z+b)/2) for s/alpha/r and
                # E=exp(z+b) for delta -- all four in exp_and_others
                for g, key in ((0, "s"), (1, "al"), (3, "r"), (2, "E")):
                    dest = E_sb if g == 2 else gt[j][key]
                    gp = gpsum.tile([F, cha], f32, tag="g")
                    for s0, sw_ in _chunks(cw, suba):
                        sl = slice(s0, s0 + sw_)
                        nc.tensor.matmul(
                            gp[:, sl], w2a[:, g * F : (g + 1) * F],
                            h1s[:, sl], start=True, stop=False,
                        )
                        nc.tensor.matmul(
                            gp[:, sl], w2b[:, g * F : (g + 1) * F],
                            h2s[:, sl], start=False, stop=True,
                        )
                    if g == 2:
                        sig_insts.append(
                            nc.scalar.activation(
                                dest[:, t0 : t0 + cw], gp[:, 0:cw],
                                AF.Exp, bias=bias2[:, 2:3],
                            )
                        )
                    else:
                        sig_insts.append(
                            nc.scalar.activation(
                                dest[:, joff : joff + cw], gp[:, 0:cw],
                                AF.Tanh, bias=bias2h[:, g : g + 1], scale=0.5,
                            )
                        )

            if b + 1 < bsh:
                prefetch_x(b + 1)

            # Total ACT order per batch (scheduler ignores sync=False hints):
            # [sigmoids] -> LoadActFuncSet(nl_exp) -> grouped ln/exp epilogue.
            # Keeps table loads at 2/batch instead of one per ln<->exp flip.
            act_chain = list(sig_insts)
            ld_inst = nc.scalar.add_instruction(
                mybir.InstLoadActFuncSet(
                    name=nc.get_next_instruction_name(),
                    act_func_set_id=NL_SET,
                    ins=[],
                    outs=[],
                )
            )
            act_chain.append(ld_inst)

            # ---- phase B: per-chunk, pipelined; chunk 0's DVE prep can
            # run while phase A still fills chunk 1's gate tiles ----
            # delta path full-tensor: the ACT chain already puts these
            # after all evacuations, so chunking them only added overhead
            dl_f = tmp.tile([F, t], bf16, tag="DL", name=f"dl_{b}")
            i_dl = nc.scalar.activation(dl_f[:], E_sb[:], AF.Ln, bias=1.0)
            ld_f = tmp.tile([F, t], bf16, tag="LD", name=f"ld_{b}")
            i_ld = nc.scalar.activation(ld_f[:], dl_f[:], AF.Ln)

            chunk_insts = []
            obs_this = []
            carry = None
            for k, (off, w) in enumerate(phb_edges):
                q = k  # distinct temp slots per chunk (no cross-chunk waits)
                gte = gt[k]

                names = iter(range(1000))

                def tl(slot, dt=bf16):
                    return tmp.tile(
                        [F, tbmax], dt, tag=f"{slot}{q}",
                        name=f"phb_{b}_{k}_{slot}{q}_{next(names)}",
                    )

                cs = slice(off, off + w)
                xck = xbuf[:, 2 + off : 2 + off + w]
                sw = slice(0, w)

                dl = dl_f[:, cs]
                ld = ld_f[:, cs]

                # tanh halves -> real gates, in place per chunk
                # (tensor_scalar runs 4x; scalar_tensor_tensor would be 1x)
                a_sb = tl("R")  # a = 1-s = 0.5 - 0.5u, straight from u
                nc.vector.tensor_scalar(
                    a_sb[:, sw], gte["s"][:], -0.5, 0.5, OP.mult, OP.add
                )
                nc.vector.tensor_scalar(
                    gte["s"][:], gte["s"][:], 0.5, 0.5, OP.mult, OP.add
                )
                nc.vector.tensor_scalar(
                    gte["al"][:], gte["al"][:], 0.5, 0.5, OP.mult, OP.add
                )
                nc.vector.tensor_scalar(
                    gte["r"][:], gte["r"][:], 0.5, 0.5, OP.mult, OP.add
                )
                bb = tl("S")
                nc.vector.tensor_tensor(bb[:, sw], gte["s"][:], xck, OP.mult)

                M = tl("M", f32)
                nc.vector.tensor_tensor_scan(
                    M[:, sw], a_sb[:, sw], bb[:, sw],
                    carry if carry is not None else 0.0,
                    OP.mult, OP.add,
                )
                carry = M[:, w - 1 : w]

                L = tl("R")  # a freed after scan; bf16 so t1 gets DVE 2x
                i_L = nc.scalar.activation(L[:, sw], M[:, sw], AF.Ln, bias=epsb[:])
                t1 = tl("S")  # bb freed after scan
                nc.vector.tensor_tensor(t1[:, sw], gte["al"][:], L[:, sw], OP.mult)
                e1 = tl("P")
                i_e1 = nc.scalar.activation(e1[:, sw], t1[:, sw], AF.Exp, scale=-1.0)
                num = tl("R")  # L freed after t1
                nc.vector.tensor_tensor(num[:, sw], xck, e1[:, sw], OP.mult)
                base = tl("S")  # t1 freed after e1
                nc.vector.tensor_tensor(base[:, sw], num[:, sw], dl, OP.add)
                lb = tl("P")  # e1 freed after num
                i_lb = nc.scalar.activation(lb[:, sw], base[:, sw], AF.Ln)
                t2 = tl("R")  # num freed after base
                nc.vector.tensor_tensor(t2[:, sw], gte["r"][:], lb[:, sw], OP.mult)
                p1 = tl("S", f32)  # base freed after lb
                i_p1 = nc.scalar.activation(p1[:, sw], t2[:, sw], AF.Exp)
                t3 = tl("R")  # t2 freed after p1
                nc.vector.tensor_tensor(t3[:, sw], gte["r"][:], ld[:, sw], OP.mult)
                p2 = tl("F2", f32)  # dl freed after base+ld
                i_p2 = nc.scalar.activation(p2[:, sw], t3[:, sw], AF.Exp)

                ob = tl("R", f32)  # t3 freed after p2
                i_ob = nc.vector.tensor_tensor(ob[:, sw], p1[:, sw], p2[:, sw], OP.subtract)
                obs_this.append(i_ob)
                nc.sync.dma_start(
                    out=out[b * F : (b + 1) * F, cs], in_=ob[:, sw]
                )
                chunk_insts.append((i_L, i_e1, i_lb, i_p1, i_p2))

            # ACT chain: per-func groups across chunks (all nl_exp set;
            # order is for pipelining only)
            act_chain.extend([i_dl, i_ld])
            for idx in range(5):
                for k in range(phb):
                    act_chain.append(chunk_insts[k][idx])
            if prev_act[0] is not None:
                add_dep_helper(
                    act_chain[0].ins, prev_act[0].ins, sync=True,
                    reason="batch act order",
                )
            for prv, nxt in zip(act_chain, act_chain[1:]):
                add_dep_helper(nxt.ins, prv.ins, sync=True, reason="act order")
            prev_act[0] = act_chain[-1]

    nc.compile()
    return nc


def _get(key=(BSH, T, CHA, SUBA, PHB)):
    if key not in _COMPILED:
        _COMPILED[key] = _build(*key)
    return _COMPILED[key]


def _in_maps(X, W1, b1, W2, b2):
    maps = []
    for i in range(N_CORES):
        maps.append(
            {
                "X": np.ascontiguousarray(
                    X[i * BSH : (i + 1) * BSH].reshape(BSH * F, T)
                ),
                "W1": np.ascontiguousarray(W1),
                "b1": np.ascontiguousarray(b1.reshape(H, 1)),
                "W2": np.ascontiguousarray(W2),
                "b2": np.ascontiguousarray(b2.reshape(4 * F, 1)),
            }
        )
    return maps


def run(X, W1, b1, W2, b2, trace=False, **kw):
    from concourse.bass_utils import run_bass_kernel_spmd

    nc = _get()
    res = run_bass_kernel_spmd(
        nc,
        _in_maps(X, W1, b1, W2, b2),
        core_ids=list(range(N_CORES)),
        trace=trace,
        **kw,
    )
    out = np.concatenate(
        [res.results[i]["out"].reshape(BSH, F, T) for i in range(N_CORES)],
        axis=0,
    ).astype(np.float32)
    return out, res


def kernel(X, W1, b1, W2, b2):
    return run(X, W1, b1, W2, b2)[0]
